# revision 31
# baseline (speedup 1.0000x reference)
"""DGCN layer (message passing GNN) on 8 Trainium2 NeuronCores via Bass/Tile.

Strategy (dst-sharded):
  - Nodes are bin-packed across the 8 cores x 49 windows of 128 dst slots;
    each core owns every edge whose dst lands in its windows, so the dst
    segment-sum is core-local.
  - The gathered table is feat = h * outdeg^-0.5 in bf16 (256B rows),
    replicated per core; per-edge rows fetched by dma_gather (SWDGE, 4
    queues, single_packet, addresses sorted ascending within each gather,
    per-(window,bucket) src dedup with trailing -1 index padding).
  - The per-edge scatter weights sel[slot, d] = sum of alpha^dist over the
    slot's merged edges are precomputed on host in fp8e4m3 (powers of two,
    near-exact) and streamed via HWDGE as dense matmul rhs blocks — no DVE
    work in the main loop.
  - Phase-1 matmuls are bf16 lhsT x fp8 rhs with fp32 PSUM accumulation;
    phase-2 is bf16 with FWL weight loads.
  - Per-node output scale s_v = indeg[v]^-3/2 applied after the W matmul;
    output streamed back in bf16 and upcast on host.

Device pipeline per core, per 128-dst window: dma_gathers fetch the
window's deduped feat[src] rows (lo/hi int16-index tables); HWDGE streams
the window's sel block; psum[f, d] += matmul(lhsT=G_tile, rhs=sel_tile)
over the window's tiles; ACT copies psum -> agg^T (bf16); then inline
phase 2: rst = matmul(lhsT=agg^T block, rhs=W) * s_v + bias -> DMA out.
"""

import math

import numpy as np

P = 128
ALPHA = 0.5
N_CORES = 8
SPLIT = 32768  # int16 index limit for dma_gather
GCH = 8  # tiles per dma_gather (finer chunks release matmuls earlier)
N_QUEUES = 4
FIRSTW = 8  # windows with full (no -1) gathers, covers pool first-use


def _wrap_idx16(flat):
    """dma_gather index layout: entry k -> partition k%16, column k//16,
    replicated across the 8 gpsimd core groups (partitions 16-127)."""
    n = flat.shape[-1]
    assert n % 16 == 0
    cols = n // 16
    w = np.asarray(flat, np.int16).reshape(cols, 16).T  # [16, cols]
    return np.tile(w, (8, 1))  # [128, cols]


def _prep_host(h, src, dst, distance, n_cores):
    """Shard edges by dst range; build per-core padded tile arrays."""
    N, D = h.shape
    E = src.shape[0]
    npc = N // n_cores
    n_windows = (npc + P - 1) // P

    src = np.asarray(src).astype(np.int64)
    dst = np.asarray(dst).astype(np.int64)
    distance = np.asarray(distance)

    out_deg = np.bincount(src, minlength=N).astype(np.float64)
    in_deg = np.bincount(dst, minlength=N).astype(np.float64)
    s_all = in_deg**-1.5  # applied after the W matmul

    # Balanced node -> (core, window, slot) assignment: deal nodes (sorted by
    # in-degree) into the n_cores*n_windows bins in rounds; within a round the
    # heaviest hi-degree nodes go to the lightest bins. This equalizes each
    # window's lo/hi edge counts, minimizing the padded tile count T (which is
    # a global max across bins). The host un-permutes output rows at the end.
    n_bins = n_cores * n_windows
    lo_deg = np.bincount(dst[src < SPLIT], minlength=N).astype(np.int64)
    hi_deg = np.bincount(dst[src >= SPLIT], minlength=N).astype(np.int64)
    order_nodes = np.argsort(-(lo_deg + hi_deg), kind="stable")
    node_bin = np.empty(N, np.int64)
    node_slot = np.empty(N, np.int64)
    lo_sum = np.zeros(n_bins, np.int64)
    hi_sum = np.zeros(n_bins, np.int64)
    fill = np.zeros(n_bins, np.int64)
    pos = 0
    while pos < N:
        take = min(n_bins, N - pos)
        nodes_r = order_nodes[pos : pos + take]
        nodes_r = nodes_r[np.argsort(-hi_deg[nodes_r], kind="stable")]
        bins_r = np.argsort(hi_sum, kind="stable")[:take]
        node_bin[nodes_r] = bins_r
        node_slot[nodes_r] = fill[bins_r]
        fill[bins_r] += 1
        lo_sum[bins_r] += lo_deg[nodes_r]
        hi_sum[bins_r] += hi_deg[nodes_r]
        pos += take
    node_core = node_bin // n_windows
    node_window = node_bin % n_windows

    core_of = node_core[dst]
    w_of = node_window[dst]
    r_of = node_slot[dst]
    is_hi = (src >= SPLIT).astype(np.int64)

    # sort edges by (core, window, lo/hi, src) — src-sorted within each
    # bucket so gather descriptors go in ascending HBM address order
    gw = (core_of * n_windows + w_of) * 2 + is_hi
    n_gw = n_cores * n_windows * 2
    counts = np.bincount(gw, minlength=n_gw)
    cl = counts.reshape(n_cores, n_windows, 2)
    T_lo = max(1, int(math.ceil(cl[:, :, 0].max() / P)))
    T_hi = max(1, int(math.ceil(cl[:, :, 1].max() / P)))
    T = T_lo + T_hi
    n_cols = n_windows * T

    order = np.lexsort((src, gw))
    sgw = gw[order]
    ssrc = src[order]
    win_start = np.concatenate([[0], np.cumsum(counts)[:-1]])

    # dedup: edges in the same (core, window, bucket) with the same src share
    # one gathered slot; their coefs sum into that slot's sel column entries
    newflag = np.ones(E, bool)
    newflag[1:] = (sgw[1:] != sgw[:-1]) | (ssrc[1:] != ssrc[:-1])
    uidx = np.cumsum(newflag) - 1  # global unique-slot counter
    grp_first_u = np.zeros(n_gw, np.int64)
    nz = counts > 0
    grp_first_u[sgw[win_start[nz]]] = uidx[win_start[nz]]
    q = uidx - grp_first_u[sgw]  # unique-slot pos within group
    ucounts = np.zeros(n_gw, np.int64)
    np.maximum.at(ucounts, sgw, q + 1)
    ucl = ucounts.reshape(n_cores, n_windows, 2)
    T_lo = max(1, int(math.ceil(ucl[:, :, 0].max() / P)))
    T_hi = max(1, int(math.ceil(ucl[:, :, 1].max() / P)))
    T = T_lo + T_hi
    n_cols = n_windows * T

    core_arr = sgw // (2 * n_windows)
    hi_arr = sgw % 2
    j_arr = q // P + hi_arr * T_lo  # hi tiles come after the lo tiles
    p_arr = q % P
    w_arr = (sgw // 2) % n_windows
    col_arr = w_arr * T + j_arr

    # dense sel blocks: sel[p, col*P + d] += alpha^dist (powers of 2)
    wvals = (np.float32(ALPHA) ** distance[order].astype(np.float32)).astype(
        np.float32
    )
    d_arr = r_of[order]
    sel = np.zeros((n_cores, P, n_cols * P), np.float32)
    np.add.at(sel, (core_arr, p_arr, col_arr * P + d_arr), wvals)

    # int16 gather indices, table-relative; slots beyond the per-core valid
    # count are 0 up to the cross-core common count, then -1 (the ucode drops
    # trailing -1s, saving descriptors and HBM traffic)
    srcrel = np.zeros((n_cores, P, n_cols), np.int64)
    srcrel[core_arr, p_arr, col_arr] = ssrc - (ssrc >= SPLIT) * SPLIT

    # per-(window, bucket, chunk) valid counts, shared across cores (the
    # gather's num_idxs_reg is baked into the SPMD program); the first FIRSTW
    # windows keep full gathers so pool buffers are never read uninitialized
    # (stale finite bf16 from 8 windows back is nullified by sel=0; fresh
    # SBUF could hold NaN bit patterns where NaN*0 would poison the psum)
    n_ck_lo = (T_lo + GCH - 1) // GCH
    n_ck_hi = (T_hi + GCH - 1) // GCH
    wmax = ucl.max(axis=0)  # [n_windows, 2]
    valid_lo = np.zeros((n_windows, n_ck_lo), np.int64)
    valid_hi = np.zeros((n_windows, n_ck_hi), np.int64)
    for w in range(n_windows):
        for k in range(n_ck_lo):
            nt = min(GCH, T_lo - k * GCH)
            full = nt * P
            valid_lo[w, k] = (
                full if w < FIRSTW else min(max(wmax[w, 0] - k * GCH * P, P), full)
            )
        for k in range(n_ck_hi):
            nt = min(GCH, T_hi - k * GCH)
            full = nt * P
            valid_hi[w, k] = (
                full if w < FIRSTW else min(max(wmax[w, 1] - k * GCH * P, P), full)
            )

    # wrapped idx16: per core, per window: lo block then hi block.
    # Blocks start at 64B-aligned column offsets (32 int16 cols).
    CL, CH = T_lo * 8, T_hi * 8  # int16 cols per window per table
    CLa = (CL + 31) // 32 * 32
    CHa = (CH + 31) // 32 * 32
    idx16 = np.zeros((n_cores, P, n_windows * (CLa + CHa)), np.int16)
    for c in range(n_cores):
        flat = srcrel[c].T  # [n_cols, P]: (tile, lane)
        for w in range(n_windows):
            lo = flat[w * T : w * T + T_lo].reshape(-1).copy()
            hi = flat[w * T + T_lo : (w + 1) * T].reshape(-1).copy()
            for k in range(n_ck_lo):
                a = k * GCH * P
                b = min(a + GCH * P, T_lo * P)
                lo[a + valid_lo[w, k] : b] = -1
            for k in range(n_ck_hi):
                a = k * GCH * P
                b = min(a + GCH * P, T_hi * P)
                hi[a + valid_hi[w, k] : b] = -1
            base = w * (CLa + CHa)
            idx16[c, :, base : base + CL] = _wrap_idx16(lo)
            idx16[c, :, base + CLa : base + CLa + CH] = _wrap_idx16(hi)

    snode = np.ones((n_cores, P, n_windows), np.float32)
    snode[node_core, node_slot, node_window] = s_all.astype(np.float32)

    # host-side inverse permutation: node v lives at core_out row
    # node_window*128 + node_slot of core node_core
    out_core = node_core
    out_row = node_window * P + node_slot

    return (
        idx16, sel, snode, out_deg, out_core, out_row,
        n_windows, T_lo, T_hi, n_cols, valid_lo, valid_hi,
    )


def _build_nc(N, D, n_windows, T_lo, T_hi, n_cols, valid_lo, valid_hi):
    import concourse.bacc as bacc
    import concourse.tile as tile
    from concourse import mybir

    f32 = mybir.dt.float32
    bf16 = mybir.dt.bfloat16
    fp8 = mybir.dt.float8e4
    i16 = mybir.dt.int16
    T = T_lo + T_hi
    CL, CH = T_lo * 8, T_hi * 8
    CLa = (CL + 31) // 32 * 32
    CHa = (CH + 31) // 32 * 32

    nc = bacc.Bacc(
        None, target_bir_lowering=False, debug=False, num_swdge_queues=N_QUEUES
    )
    h_d = nc.declare_dram_parameter("h16", [N, D], bf16, isOutput=False)
    idx_d = nc.declare_dram_parameter(
        "idx16", [P, n_windows * (CLa + CHa)], i16, isOutput=False
    )
    sel_d = nc.declare_dram_parameter("sel8", [P, n_cols * P], fp8, isOutput=False)
    w_d = nc.declare_dram_parameter("w16", [P, D], bf16, isOutput=False)
    fc_d = nc.declare_dram_parameter(
        "fconst", [P, D + n_windows], f32, isOutput=False
    )
    out_d = nc.declare_dram_parameter("out", [n_windows * P, D], bf16, isOutput=True)

    mult = mybir.AluOpType.mult

    with tile.TileContext(nc) as tc:
        with (
            tc.tile_pool(name="singles", bufs=1) as singles,
            tc.tile_pool(name="glo", bufs=8) as glopool,
            tc.tile_pool(name="ghi", bufs=8) as ghipool,
            tc.tile_pool(name="sel", bufs=8) as selpool,
            tc.tile_pool(name="psum", bufs=4, space="PSUM") as psumpool,
            tc.tile_pool(name="psum2", bufs=2, space="PSUM") as psum2pool,
            tc.tile_pool(name="outp", bufs=3) as outpool,
        ):
            idx_sb = singles.tile([P, n_windows * (CLa + CHa)], i16)
            tot = n_windows * (CLa + CHa)
            hd = min(4, n_windows) * (CLa + CHa)
            nc.sync.dma_start(out=idx_sb[:, :hd], in_=idx_d[:, :hd])
            if hd < tot:
                nc.sync.dma_start(out=idx_sb[:, hd:], in_=idx_d[:, hd:])
            w_sb = singles.tile([P, D], bf16)
            nc.sync.dma_start(out=w_sb[:], in_=w_d[:])
            fc_sb = singles.tile([P, D + n_windows], f32)
            nc.sync.dma_start(out=fc_sb[:], in_=fc_d[:])

            b_sb = fc_sb[:, 0:D]
            s_sb = fc_sb[:, D : D + n_windows]

            agg = singles.tile([P, n_windows * P], bf16)  # agg^T [feat, node]

            def _phase2(w2):
                ps2 = psum2pool.tile([P, D], f32)
                nc.tensor.matmul(
                    out=ps2[:],
                    lhsT=agg[:, w2 * P : (w2 + 1) * P],
                    rhs=w_sb,
                    start=True,
                    stop=True,
                )
                o = outpool.tile([P, D], bf16)
                ot = outpool.tile([P, D], f32, tag="ot")
                nc.vector.tensor_tensor(
                    out=ot[:],
                    in0=ps2[:],
                    in1=s_sb[:, w2 : w2 + 1].to_broadcast([P, D]),
                    op=mult,
                )
                nc.vector.tensor_add(out=o[:], in0=ot[:], in1=b_sb)
                nc.sync.dma_start(out=out_d[w2 * P : (w2 + 1) * P, :], in_=o[:])

            h_lo = h_d[0 : min(SPLIT, N), :]
            hi_base = SPLIT if N > SPLIT else 0
            h_hi = h_d[hi_base:N, :]

            qctr = 0
            for w in range(n_windows):
                base = w * (CLa + CHa)
                lo_chunks = []
                for k in range((T_lo + GCH - 1) // GCH):
                    nt = min(GCH, T_lo - k * GCH)
                    g = glopool.tile([P, GCH, P], bf16, tag="glo")
                    cb = base + k * GCH * 8
                    nc.gpsimd.dma_gather(
                        g[:, :nt, :],
                        h_lo,
                        idx_sb[:, cb : cb + nt * 8],
                        nt * P,
                        int(valid_lo[w, k]),
                        P,
                        single_packet=True,
                        queue_num=qctr % N_QUEUES,
                    )
                    qctr += 1
                    lo_chunks.append(g)
                hi_chunks = []
                for k in range((T_hi + GCH - 1) // GCH):
                    nt = min(GCH, T_hi - k * GCH)
                    g = ghipool.tile([P, GCH, P], bf16, tag="ghi")
                    cb = base + CLa + k * GCH * 8
                    nc.gpsimd.dma_gather(
                        g[:, :nt, :],
                        h_hi,
                        idx_sb[:, cb : cb + nt * 8],
                        nt * P,
                        int(valid_hi[w, k]),
                        P,
                        single_packet=True,
                        queue_num=qctr % N_QUEUES,
                    )
                    qctr += 1
                    hi_chunks.append(g)
                sel_sb = selpool.tile([P, T * P], fp8)
                nc.sync.dma_start(
                    out=sel_sb[:], in_=sel_d[:, w * T * P : (w + 1) * T * P]
                )
                ps = psumpool.tile([P, P], f32)
                for j in range(T):
                    if j < T_lo:
                        lhsT = lo_chunks[j // GCH][:, j % GCH, :]
                    else:
                        jh = j - T_lo
                        lhsT = hi_chunks[jh // GCH][:, jh % GCH, :]
                    nc.tensor.matmul(
                        out=ps[:],
                        lhsT=lhsT,
                        rhs=sel_sb[:, j * P : (j + 1) * P],
                        start=(j == 0),
                        stop=(j == T - 1),
                    )
                nc.scalar.copy(out=agg[:, w * P : (w + 1) * P], in_=ps[:])
                # phase 2 inline: the window's output work hides in the
                # gather shadow of subsequent windows
                _phase2(w)

    nc.compile()
    return nc


def kernel(h, src, dst, distance, weight, bias, _trace=False):
    import ml_dtypes

    from concourse.bass_utils import run_bass_kernel_spmd

    bf16 = ml_dtypes.bfloat16
    fp8 = ml_dtypes.float8_e4m3

    h = np.ascontiguousarray(np.asarray(h, dtype=np.float32))
    weight = np.asarray(weight, dtype=np.float32)
    bias = np.asarray(bias, dtype=np.float32)
    N, D = h.shape

    (
        idx16, sel, snode, out_deg, out_core, out_row,
        n_windows, T_lo, T_hi, n_cols, valid_lo, valid_hi,
    ) = _prep_host(h, src, dst, distance, N_CORES)

    # gathered table: source-side normalized features, bf16 rows (256B)
    feat16 = np.ascontiguousarray(
        (h * (out_deg**-0.5)[:, None].astype(np.float32)).astype(bf16)
    )
    w16 = np.ascontiguousarray(weight.astype(bf16))
    biasf = np.broadcast_to(bias[None, :], (P, D))

    nc = _build_nc(N, D, n_windows, T_lo, T_hi, n_cols, valid_lo, valid_hi)

    in_maps = []
    for c in range(N_CORES):
        fconst = np.ascontiguousarray(
            np.concatenate([biasf, snode[c]], axis=1).astype(np.float32)
        )
        in_maps.append(
            {
                "h16": feat16,
                "idx16": np.ascontiguousarray(idx16[c]),
                "sel8": np.ascontiguousarray(sel[c].astype(fp8)),
                "w16": w16,
                "fconst": fconst,
            }
        )

    res = run_bass_kernel_spmd(nc, in_maps, list(range(N_CORES)), trace=_trace)

    stacked = np.stack(
        [np.asarray(res.results[c]["out"]).astype(np.float32) for c in range(N_CORES)]
    )
    out = stacked[out_core, out_row].astype(np.float32)

    if _trace:
        return out, res
    return out


# revision 38
# speedup vs baseline: 1.0446x; 1.0446x over previous
"""DGCN layer (message passing GNN) on 8 Trainium2 NeuronCores via Bass/Tile.

Strategy (dst-sharded):
  - Nodes are bin-packed across the 8 cores x 49 windows of 128 dst slots;
    each core owns every edge whose dst lands in its windows, so the dst
    segment-sum is core-local.
  - The gathered table is feat = h * outdeg^-0.5 in bf16 (256B rows),
    replicated per core; per-edge rows fetched by dma_gather (SWDGE, 4
    queues, single_packet, addresses sorted ascending within each gather,
    per-(window,bucket) src dedup with trailing -1 index padding).
  - The per-edge scatter weights sel[slot, d] = sum of alpha^dist over the
    slot's merged edges are precomputed on host in fp8e4m3 (powers of two,
    near-exact) and streamed via HWDGE as dense matmul rhs blocks — no DVE
    work in the main loop.
  - Phase-1 matmuls are bf16 lhsT x fp8 rhs with fp32 PSUM accumulation;
    phase-2 is bf16 with FWL weight loads.
  - Per-node output scale s_v = indeg[v]^-3/2 applied after the W matmul;
    output streamed back in bf16 and upcast on host.

Device pipeline per core, per 128-dst window: dma_gathers fetch the
window's deduped feat[src] rows (lo/hi int16-index tables); HWDGE streams
the window's sel block; psum[f, d] += matmul(lhsT=G_tile, rhs=sel_tile)
over the window's tiles; ACT copies psum -> agg^T (bf16); then inline
phase 2: rst = matmul(lhsT=agg^T block, rhs=W) * s_v + bias -> DMA out.
"""

import math

import numpy as np

P = 128
ALPHA = 0.5
N_CORES = 8
SPLIT = 32768  # int16 index limit for dma_gather
GCH = 8  # tiles per dma_gather (finer chunks release matmuls earlier)
N_QUEUES = 4
FIRSTW = 8  # windows with full (no -1) gathers, covers pool first-use


def _wrap_idx16(flat):
    """dma_gather index layout: entry k -> partition k%16, column k//16,
    replicated across the 8 gpsimd core groups (partitions 16-127)."""
    n = flat.shape[-1]
    assert n % 16 == 0
    cols = n // 16
    w = np.asarray(flat, np.int16).reshape(cols, 16).T  # [16, cols]
    return np.tile(w, (8, 1))  # [128, cols]


def _prep_host(h, src, dst, distance, n_cores):
    """Shard edges by dst range; build per-core padded tile arrays."""
    N, D = h.shape
    E = src.shape[0]
    npc = N // n_cores
    n_windows = (npc + P - 1) // P

    src = np.asarray(src).astype(np.int64)
    dst = np.asarray(dst).astype(np.int64)
    distance = np.asarray(distance)

    out_deg = np.bincount(src, minlength=N).astype(np.float64)
    in_deg = np.bincount(dst, minlength=N).astype(np.float64)
    s_all = in_deg**-1.5  # applied after the W matmul

    # Balanced node -> (core, window, slot) assignment: deal nodes (sorted by
    # in-degree) into the n_cores*n_windows bins in rounds; within a round the
    # heaviest hi-degree nodes go to the lightest bins. This equalizes each
    # window's lo/hi edge counts, minimizing the padded tile count T (which is
    # a global max across bins). The host un-permutes output rows at the end.
    n_bins = n_cores * n_windows
    lo_deg = np.bincount(dst[src < SPLIT], minlength=N).astype(np.int64)
    hi_deg = np.bincount(dst[src >= SPLIT], minlength=N).astype(np.int64)
    order_nodes = np.argsort(-(lo_deg + hi_deg), kind="stable")
    node_bin = np.empty(N, np.int64)
    node_slot = np.empty(N, np.int64)
    lo_sum = np.zeros(n_bins, np.int64)
    hi_sum = np.zeros(n_bins, np.int64)
    fill = np.zeros(n_bins, np.int64)
    pos = 0
    while pos < N:
        take = min(n_bins, N - pos)
        nodes_r = order_nodes[pos : pos + take]
        nodes_r = nodes_r[np.argsort(-hi_deg[nodes_r], kind="stable")]
        bins_r = np.argsort(hi_sum, kind="stable")[:take]
        node_bin[nodes_r] = bins_r
        node_slot[nodes_r] = fill[bins_r]
        fill[bins_r] += 1
        lo_sum[bins_r] += lo_deg[nodes_r]
        hi_sum[bins_r] += hi_deg[nodes_r]
        pos += take
    node_core = node_bin // n_windows
    node_window = node_bin % n_windows

    core_of = node_core[dst]
    w_of = node_window[dst]
    r_of = node_slot[dst]
    is_hi = (src >= SPLIT).astype(np.int64)

    # sort edges by (core, window, lo/hi, src) — src-sorted within each
    # bucket so gather descriptors go in ascending HBM address order
    gw = (core_of * n_windows + w_of) * 2 + is_hi
    n_gw = n_cores * n_windows * 2
    counts = np.bincount(gw, minlength=n_gw)
    cl = counts.reshape(n_cores, n_windows, 2)
    T_lo = max(1, int(math.ceil(cl[:, :, 0].max() / P)))
    T_hi = max(1, int(math.ceil(cl[:, :, 1].max() / P)))
    T = T_lo + T_hi
    n_cols = n_windows * T

    order = np.lexsort((src, gw))
    sgw = gw[order]
    ssrc = src[order]
    win_start = np.concatenate([[0], np.cumsum(counts)[:-1]])

    # dedup: edges in the same (core, window, bucket) with the same src share
    # one gathered slot; their coefs sum into that slot's sel column entries
    newflag = np.ones(E, bool)
    newflag[1:] = (sgw[1:] != sgw[:-1]) | (ssrc[1:] != ssrc[:-1])
    uidx = np.cumsum(newflag) - 1  # global unique-slot counter
    grp_first_u = np.zeros(n_gw, np.int64)
    nz = counts > 0
    grp_first_u[sgw[win_start[nz]]] = uidx[win_start[nz]]
    q = uidx - grp_first_u[sgw]  # unique-slot pos within group
    ucounts = np.zeros(n_gw, np.int64)
    np.maximum.at(ucounts, sgw, q + 1)
    ucl = ucounts.reshape(n_cores, n_windows, 2)
    T_lo = max(1, int(math.ceil(ucl[:, :, 0].max() / P)))
    T_hi = max(1, int(math.ceil(ucl[:, :, 1].max() / P)))
    T = T_lo + T_hi
    n_cols = n_windows * T

    core_arr = sgw // (2 * n_windows)
    hi_arr = sgw % 2
    j_arr = q // P + hi_arr * T_lo  # hi tiles come after the lo tiles
    p_arr = q % P
    w_arr = (sgw // 2) % n_windows
    col_arr = w_arr * T + j_arr

    # dense sel blocks: sel[p, col*P + d] += alpha^dist (powers of 2)
    wvals = (np.float32(ALPHA) ** distance[order].astype(np.float32)).astype(
        np.float32
    )
    d_arr = r_of[order]
    sel = np.zeros((n_cores, P, n_cols * P), np.float32)
    np.add.at(sel, (core_arr, p_arr, col_arr * P + d_arr), wvals)

    # int16 gather indices, table-relative; slots beyond the per-core valid
    # count are 0 up to the cross-core common count, then -1 (the ucode drops
    # trailing -1s, saving descriptors and HBM traffic)
    srcrel = np.zeros((n_cores, P, n_cols), np.int64)
    srcrel[core_arr, p_arr, col_arr] = ssrc - (ssrc >= SPLIT) * SPLIT

    # per-(window, bucket) gathered-tile counts, shared across cores (the
    # SPMD program is identical on every core, so use the cross-core max of
    # the deduped counts). Tiles beyond the count are never gathered: their
    # sel columns are all-zero, and the matmul reads stale-but-finite data.
    # The first FIRSTW windows gather every tile so pool buffers are never
    # read uninitialized (fresh SBUF could hold NaN patterns; NaN*0 = NaN
    # would poison the psum — stale bf16 features from 8 windows back are
    # always finite).
    wmax = ucl.max(axis=0)  # [n_windows, 2]
    ntl = np.minimum(np.maximum((wmax[:, 0] + P - 1) // P, 1), T_lo)
    nth = np.minimum(np.maximum((wmax[:, 1] + P - 1) // P, 1), T_hi)
    ntl[:FIRSTW] = T_lo
    nth[:FIRSTW] = T_hi

    # wrapped idx16: per core, per window: lo block then hi block.
    # Blocks start at 64B-aligned column offsets (32 int16 cols).
    CL, CH = T_lo * 8, T_hi * 8  # int16 cols per window per table
    CLa = (CL + 31) // 32 * 32
    CHa = (CH + 31) // 32 * 32
    idx16 = np.zeros((n_cores, P, n_windows * (CLa + CHa)), np.int16)
    for c in range(n_cores):
        flat = srcrel[c].T  # [n_cols, P]: (tile, lane)
        for w in range(n_windows):
            lo = flat[w * T : w * T + T_lo].reshape(-1)
            hi = flat[w * T + T_lo : (w + 1) * T].reshape(-1)
            base = w * (CLa + CHa)
            idx16[c, :, base : base + CL] = _wrap_idx16(lo)
            idx16[c, :, base + CLa : base + CLa + CH] = _wrap_idx16(hi)

    snode = np.ones((n_cores, P, n_windows), np.float32)
    snode[node_core, node_slot, node_window] = s_all.astype(np.float32)

    # host-side inverse permutation: node v lives at core_out row
    # node_window*128 + node_slot of core node_core
    out_core = node_core
    out_row = node_window * P + node_slot

    return (
        idx16, sel, snode, out_deg, out_core, out_row,
        n_windows, T_lo, T_hi, n_cols, ntl, nth,
    )


def _build_nc(N, D, n_windows, T_lo, T_hi, n_cols, ntl, nth):
    import concourse.bacc as bacc
    import concourse.tile as tile
    from concourse import mybir

    f32 = mybir.dt.float32
    bf16 = mybir.dt.bfloat16
    fp8 = mybir.dt.float8e4
    i16 = mybir.dt.int16
    T = T_lo + T_hi
    CL, CH = T_lo * 8, T_hi * 8
    CLa = (CL + 31) // 32 * 32
    CHa = (CH + 31) // 32 * 32

    nc = bacc.Bacc(
        None, target_bir_lowering=False, debug=False, num_swdge_queues=N_QUEUES
    )
    h_d = nc.declare_dram_parameter("h16", [N, D], bf16, isOutput=False)
    idx_d = nc.declare_dram_parameter(
        "idx16", [P, n_windows * (CLa + CHa)], i16, isOutput=False
    )
    sel_d = nc.declare_dram_parameter("sel8", [P, n_cols * P], fp8, isOutput=False)
    w_d = nc.declare_dram_parameter("w16", [P, D], bf16, isOutput=False)
    fc_d = nc.declare_dram_parameter(
        "fconst", [P, D + n_windows], f32, isOutput=False
    )
    out_d = nc.declare_dram_parameter("out", [n_windows * P, D], bf16, isOutput=True)

    mult = mybir.AluOpType.mult

    with tile.TileContext(nc) as tc:
        with (
            tc.tile_pool(name="singles", bufs=1) as singles,
            tc.tile_pool(name="glo", bufs=8) as glopool,
            tc.tile_pool(name="ghi", bufs=8) as ghipool,
            tc.tile_pool(name="sel", bufs=8) as selpool,
            tc.tile_pool(name="psum", bufs=4, space="PSUM") as psumpool,
            tc.tile_pool(name="psum2", bufs=2, space="PSUM") as psum2pool,
            tc.tile_pool(name="outp", bufs=3) as outpool,
        ):
            idx_sb = singles.tile([P, n_windows * (CLa + CHa)], i16)
            tot = n_windows * (CLa + CHa)
            hd = min(4, n_windows) * (CLa + CHa)
            nc.sync.dma_start(out=idx_sb[:, :hd], in_=idx_d[:, :hd])
            if hd < tot:
                nc.sync.dma_start(out=idx_sb[:, hd:], in_=idx_d[:, hd:])
            w_sb = singles.tile([P, D], bf16)
            nc.sync.dma_start(out=w_sb[:], in_=w_d[:])
            fc_sb = singles.tile([P, D + n_windows], f32)
            nc.sync.dma_start(out=fc_sb[:], in_=fc_d[:])

            b_sb = fc_sb[:, 0:D]
            s_sb = fc_sb[:, D : D + n_windows]

            agg = singles.tile([P, n_windows * P], bf16)  # agg^T [feat, node]

            def _phase2(w2):
                ps2 = psum2pool.tile([P, D], f32)
                nc.tensor.matmul(
                    out=ps2[:],
                    lhsT=agg[:, w2 * P : (w2 + 1) * P],
                    rhs=w_sb,
                    start=True,
                    stop=True,
                )
                o = outpool.tile([P, D], bf16)
                ot = outpool.tile([P, D], f32, tag="ot")
                nc.vector.tensor_tensor(
                    out=ot[:],
                    in0=ps2[:],
                    in1=s_sb[:, w2 : w2 + 1].to_broadcast([P, D]),
                    op=mult,
                )
                nc.vector.tensor_add(out=o[:], in0=ot[:], in1=b_sb)
                nc.sync.dma_start(out=out_d[w2 * P : (w2 + 1) * P, :], in_=o[:])

            h_lo = h_d[0 : min(SPLIT, N), :]
            hi_base = SPLIT if N > SPLIT else 0
            h_hi = h_d[hi_base:N, :]

            qctr = 0
            for w in range(n_windows):
                base = w * (CLa + CHa)
                lo_chunks = []
                for k in range((int(ntl[w]) + GCH - 1) // GCH):
                    nt = min(GCH, int(ntl[w]) - k * GCH)
                    g = glopool.tile([P, GCH, P], bf16, tag="glo")
                    cb = base + k * GCH * 8
                    nc.gpsimd.dma_gather(
                        g[:, :nt, :],
                        h_lo,
                        idx_sb[:, cb : cb + nt * 8],
                        nt * P,
                        nt * P,
                        P,
                        single_packet=True,
                        queue_num=qctr % N_QUEUES,
                    )
                    qctr += 1
                    lo_chunks.append(g)
                hi_chunks = []
                for k in range((int(nth[w]) + GCH - 1) // GCH):
                    nt = min(GCH, int(nth[w]) - k * GCH)
                    g = ghipool.tile([P, GCH, P], bf16, tag="ghi")
                    cb = base + CLa + k * GCH * 8
                    nc.gpsimd.dma_gather(
                        g[:, :nt, :],
                        h_hi,
                        idx_sb[:, cb : cb + nt * 8],
                        nt * P,
                        nt * P,
                        P,
                        single_packet=True,
                        queue_num=qctr % N_QUEUES,
                    )
                    qctr += 1
                    hi_chunks.append(g)
                sel_sb = selpool.tile([P, T * P], fp8)
                nc.sync.dma_start(
                    out=sel_sb[:], in_=sel_d[:, w * T * P : (w + 1) * T * P]
                )
                ps = psumpool.tile([P, P], f32)
                # tiles beyond the window's gathered count have all-zero sel
                # columns; skip their matmuls entirely
                live = [(j, lo_chunks[j // GCH][:, j % GCH, :])
                        for j in range(int(ntl[w]))]
                live += [(T_lo + j, hi_chunks[j // GCH][:, j % GCH, :])
                         for j in range(int(nth[w]))]
                for i, (j, lhsT) in enumerate(live):
                    nc.tensor.matmul(
                        out=ps[:],
                        lhsT=lhsT,
                        rhs=sel_sb[:, j * P : (j + 1) * P],
                        start=(i == 0),
                        stop=(i == len(live) - 1),
                    )
                nc.scalar.copy(out=agg[:, w * P : (w + 1) * P], in_=ps[:])
                # phase 2 inline: the window's output work hides in the
                # gather shadow of subsequent windows
                _phase2(w)

    nc.compile()
    return nc


def kernel(h, src, dst, distance, weight, bias, _trace=False):
    import ml_dtypes

    from concourse.bass_utils import run_bass_kernel_spmd

    bf16 = ml_dtypes.bfloat16
    fp8 = ml_dtypes.float8_e4m3

    h = np.ascontiguousarray(np.asarray(h, dtype=np.float32))
    weight = np.asarray(weight, dtype=np.float32)
    bias = np.asarray(bias, dtype=np.float32)
    N, D = h.shape

    (
        idx16, sel, snode, out_deg, out_core, out_row,
        n_windows, T_lo, T_hi, n_cols, ntl, nth,
    ) = _prep_host(h, src, dst, distance, N_CORES)

    # gathered table: source-side normalized features, bf16 rows (256B)
    feat16 = np.ascontiguousarray(
        (h * (out_deg**-0.5)[:, None].astype(np.float32)).astype(bf16)
    )
    w16 = np.ascontiguousarray(weight.astype(bf16))
    biasf = np.broadcast_to(bias[None, :], (P, D))

    nc = _build_nc(N, D, n_windows, T_lo, T_hi, n_cols, ntl, nth)

    in_maps = []
    for c in range(N_CORES):
        fconst = np.ascontiguousarray(
            np.concatenate([biasf, snode[c]], axis=1).astype(np.float32)
        )
        in_maps.append(
            {
                "h16": feat16,
                "idx16": np.ascontiguousarray(idx16[c]),
                "sel8": np.ascontiguousarray(sel[c].astype(fp8)),
                "w16": w16,
                "fconst": fconst,
            }
        )

    res = run_bass_kernel_spmd(nc, in_maps, list(range(N_CORES)), trace=_trace)

    stacked = np.stack(
        [np.asarray(res.results[c]["out"]).astype(np.float32) for c in range(N_CORES)]
    )
    out = stacked[out_core, out_row].astype(np.float32)

    if _trace:
        return out, res
    return out
